# revision 11
# baseline (speedup 1.0000x reference)
"""MoE router (top-2 gating) Trainium2 Bass kernel, SPMD over 8 NeuronCores.

Problem: x [4, 4096, 2048] f32, gate_w [64, 2048] f32.
  logits = x @ gate_w.T          -> [4, 4096, 64]
  scores, indices = top_k(logits, 2)
  weights = softmax(scores)      -> ([4, 4096, 2] f32, [4, 4096, 2] i32)

Sharding: data-parallel over tokens; each of the 8 cores gets 2048 tokens,
shipped transposed ([D, T]) so the contraction dim D lands on SBUF
partitions and the PE streams tokens as the moving operand of exact-fp32
matmuls (min top2/top3 logit gap on this input is 4e-7 — any reduced
precision flips indices, so the matmul must stay fp32).

Schedule (v3, from trace analysis):
  - ALL 8 input sub-DMAs issued first on the sync HWDGE ring; gw + output
    DMAs ride the scalar HWDGE ring (v1 stalled the input stream ~8us
    behind an output DMA waiting on softmax).
  - Every input piece keeps a >=256-token extent so DRAM rows stay >=1KB
    (fine token splits exploded the descriptor count in v2 and made the
    stream issue-bound: 2048-desc pieces took 8-12us to generate).
  - Pieces: 4x(4dt x tok[0:1024]), 2x(6dt x tok[1024:2048]),
    (4dt x tok[1024:1792]), (4dt x tok[1792:2048]).  The last piece is
    0.5 MiB so only 256 tokens' worth of top-k trails the stream.
  - Compute chunks [1024, 768, 256] slice into those pieces; chunk 1+2
    accumulate interleaved per d-tile as B pieces land.
  - PE warm-up matmuls lift the HAM clock gate before real work.
  - softmax(top2) via two ACT sigmoids (w1 = sig(d), w0 = sig(-d)).
  - index-half output DMAs go on the (idle) sync ring as soon as indices
    are compacted; weight halves follow sigmoids on the scalar ring.
"""
import sys

if "/opt/trn_rl_repo" not in sys.path:
    sys.path.insert(0, "/opt/trn_rl_repo")

import numpy as np

B, T, D, E, K = 4, 4096, 2048, 64, 2
N_CORES = 8
P = 128
NDT = D // P                      # 16 d-tiles
TOK_PER_CORE = B * T // N_CORES   # 2048
NSEG = TOK_PER_CORE // P          # 16 output segments of 128 tokens

# input pieces: (name, d0, ndt, t0, ntok), issued in this order.
# v5: fewer, larger pieces (fewer DMA semaphores -> shorter framework
# preamble/teardown); chunk1 finishes 2 pieces before stream end so its
# epilogue overlaps the stream; trailing pieces shrink so the post-stream
# tail is only 4 d-tiles of 512 tokens + one epilogue.
PIECES = [
    ("a0", 0, 2, 0, 1024),
    ("a1", 2, 2, 0, 1024),
    ("a2", 4, 4, 0, 1024),
    ("a3", 8, 4, 0, 1024),
    ("a4", 12, 4, 0, 1024),
    ("b0", 0, 8, 1024, 512),
    ("b1", 8, 8, 1024, 512),
    ("c0", 0, 8, 1536, 512),
    ("c1", 8, 4, 1536, 512),
    ("c2", 12, 4, 1536, 512),
]
# compute chunks: (t0, ntok); chunk k completes when its last piece lands,
# staggered so each epilogue overlaps the remaining stream
CHUNKS = [(0, 1024), (1024, 512), (1536, 512)]
N_WARMUP = 10          # PE warm-up matmuls before the first real one

_compiled = None


def _build():
    import concourse.bacc as bacc
    import concourse.tile as tile
    from concourse import mybir
    from concourse.masks import make_identity

    nc = bacc.Bacc("TRN2", target_bir_lowering=False, debug=False,
                   num_devices=N_CORES)

    # packed stream-order layout: pieces concatenated along the free dim,
    # each piece stored [p, dt*ntok + t] — every sub-DMA reads a fully
    # contiguous per-partition run (4-16 KB) instead of 4 KB strided rows
    xT_in = nc.dram_tensor("xT", [P, NDT * TOK_PER_CORE], mybir.dt.float32,
                           kind="ExternalInput")
    gw_in = nc.dram_tensor("gwl", [P, NDT * E], mybir.dt.float32,
                           kind="ExternalInput")
    # single merged output: [:, 0:NSEG*K] = weight bits (f32), rest = indices
    o_out = nc.dram_tensor("o", [P, NSEG * K * 2], mybir.dt.uint32,
                           kind="ExternalOutput")

    fp32 = mybir.dt.float32

    with tile.TileContext(nc) as tc:
        with (
            tc.tile_pool(name="xpool", bufs=1) as xpool,
            tc.tile_pool(name="cpool", bufs=1) as cpool,
            tc.tile_pool(name="epool", bufs=2) as epool,
            tc.tile_pool(name="opool", bufs=1) as opool,
            tc.tile_pool(name="pacc", bufs=2, space="PSUM") as pacc,
            tc.tile_pool(name="plg", bufs=2, space="PSUM") as plg,
            tc.tile_pool(name="pwarm", bufs=1, space="PSUM") as pwarm,
        ):
            # ---- input stream: every sub-DMA issued first, sync ring ----
            pieces = {}   # name -> (d0, ndt, t0, ntok, tile)
            off = 0
            for (nm, d0, nd, t0, ntok) in PIECES:
                xt = xpool.tile([P, nd * ntok], fp32,
                                tag=f"x_{nm}", name=f"xt_{nm}")
                nc.sync.dma_start(
                    xt[:], xT_in.ap()[:, off:off + nd * ntok])
                off += nd * ntok
                pieces[nm] = (d0, nd, t0, ntok, xt)

            def src_ap(dt, ct0, cn):
                """moving operand slice for d-tile dt, tokens [ct0, ct0+cn)"""
                for (d0, nd, t0, ntok, xt) in pieces.values():
                    if d0 <= dt < d0 + nd and t0 <= ct0 and ct0 + cn <= t0 + ntok:
                        base = (dt - d0) * ntok + (ct0 - t0)
                        return xt[:, base:base + cn]
                raise AssertionError((dt, ct0, cn))

            # ---- constants / one-time loads (scalar ring for gw) ----
            # split so the dt0-3 slice lands before the first matmul needs
            # it (scalar-ring packets interleave 1:1 with the input flood,
            # so the full 512KB otherwise gates the PE until ~13.3us)
            gw_sb = cpool.tile([P, NDT * E], fp32)
            nc.scalar.dma_start(gw_sb[:, 0:4 * E], gw_in.ap()[:, 0:4 * E])
            nc.scalar.dma_start(gw_sb[:, 4 * E:], gw_in.ap()[:, 4 * E:])
            ident = cpool.tile([P, P], fp32)
            make_identity(nc, ident[:])
            # warm the ACT sigmoid table early (overlaps input stream)
            scratch = cpool.tile([P, 1], fp32)
            nc.gpsimd.memset(scratch[:], 0.0)
            nc.scalar.activation(scratch[:], scratch[:],
                                 mybir.ActivationFunctionType.Sigmoid)

            # ---- PE warm-up: lift the HAM clock gate before real work ----
            warm = pwarm.tile([P, P], fp32, tag="warm", name="warm")
            for wi in range(N_WARMUP):
                nc.tensor.matmul(warm[:], ident[:], ident[:],
                                 start=True, stop=True)

            # ---- per-core accumulators ----
            mx_acc = opool.tile([P, NSEG * 8], fp32)
            mi_acc = opool.tile([P, NSEG * 8], mybir.dt.uint32)
            acc_all = opool.tile([P, NSEG * K * 2], mybir.dt.uint32)
            mx3 = mx_acc[:].rearrange("p (s k) -> p s k", k=8)
            wv = acc_all[:, 0:NSEG * K].bitcast(fp32).rearrange(
                "p (s k) -> p s k", k=K)
            mi3 = mi_acc[:].rearrange("p (s k) -> p s k", k=8)

            def emit_mm(ci, ct0, cn, pga, pgb, dts):
                half = cn // 2
                for dt in dts:
                    gsl = gw_sb[:, dt * E:(dt + 1) * E]
                    mmargs = dict(start=(dt == 0), stop=(dt == NDT - 1))
                    nc.tensor.matmul(pga[:, :half], gsl,
                                     src_ap(dt, ct0, half),
                                     tile_position=(0, 0), **mmargs)
                    nc.tensor.matmul(pgb[64:128, :half], gsl,
                                     src_ap(dt, ct0 + half, half),
                                     tile_position=(0, 64), **mmargs)

            def emit_mm_half(quad, ct0, pq, dts, stop_dt):
                # one 512-token half packed into PE column quadrant `quad`;
                # its accumulation group stops at its own last d-tile
                for dt in dts:
                    gsl = gw_sb[:, dt * E:(dt + 1) * E]
                    nc.tensor.matmul(
                        pq, gsl, src_ap(dt, ct0, 512),
                        tile_position=(0, quad),
                        start=(dt == 0), stop=(dt == stop_dt))

            def emit_epilogue_half(ci, ct0, pq, rowlo, cb):
                # epilogue for a 512-token half living in PSUM rows
                # [rowlo, rowlo+64); experts land at cols [cb, cb+64)
                s0 = ct0 // P
                lt = epool.tile([P, 1024], fp32, tag="lt", name=f"lt{ci}")
                cp = nc.vector.tensor_copy if rowlo == 0 else nc.scalar.copy
                cp(lt[rowlo:rowlo + 64, 0:512], pq)
                lg_ps = plg.tile([P, 512], fp32, tag="lg_ps",
                                 name=f"lgps{ci}")
                for j in range(4):
                    nc.tensor.transpose(
                        lg_ps[:, j * P:(j + 1) * P],
                        lt[:, j * P:(j + 1) * P], ident[:],
                    )
                for j in range(4):
                    s = s0 + j
                    nc.vector.max(
                        out=mx_acc[:, s * 8:(s + 1) * 8],
                        in_=lg_ps[:, j * P + cb: j * P + cb + 64])
                s1, nsg = s0 + 4, 4
                delta = epool.tile([P, 16], fp32, tag="delta",
                                   name=f"delta{ci}")
                nc.vector.tensor_tensor(delta[:, :nsg], mx3[:, s0:s1, 1],
                                        mx3[:, s0:s1, 0],
                                        op=mybir.AluOpType.subtract)
                nc.scalar.activation(wv[:, s0:s1, 1], delta[:, :nsg],
                                     mybir.ActivationFunctionType.Sigmoid)
                nc.scalar.activation(wv[:, s0:s1, 0], delta[:, :nsg],
                                     mybir.ActivationFunctionType.Sigmoid,
                                     scale=-1.0)
                nc.scalar.dma_start(o_out.ap()[:, s0 * K:s1 * K],
                                    acc_all[:, s0 * K:s1 * K])
                for j in range(4):
                    s = s0 + j
                    nc.vector.max_index(
                        mi_acc[:, s * 8:(s + 1) * 8],
                        mx_acc[:, s * 8:(s + 1) * 8],
                        lg_ps[:, j * P + cb: j * P + cb + 64])
                nc.gpsimd.tensor_copy(
                    acc_all[:, NSEG * K + s0 * K: NSEG * K + s1 * K]
                    .rearrange("p (s k) -> p s k", k=K),
                    mi3[:, s0:s1, 0:K])
                nc.sync.dma_start(
                    o_out.ap()[:, NSEG * K + s0 * K:NSEG * K + s1 * K],
                    acc_all[:, NSEG * K + s0 * K:NSEG * K + s1 * K])

            def emit_epilogue(ci, ct0, cn, pga, pgb):
                half = cn // 2
                nblk = cn // P
                s0 = ct0 // P
                # copy the two logits.T halves into token-aligned quadrants
                # (gpsimd can't read PSUM, so DVE + scalar)
                lt = epool.tile([P, 1024], fp32, tag="lt", name=f"lt{ci}")
                nc.vector.tensor_copy(lt[0:64, 0:half], pga[:, :half])
                nc.scalar.copy(lt[64:128, half:cn], pgb[64:128, :half])
                # back-transpose in passes of <=4 blocks (plg bank = 512 f32);
                # top-8 / top-8-index read the transposed PSUM directly
                segs = {}
                for pi in range(0, nblk, 4):
                    pe = min(pi + 4, nblk)
                    last = pe == nblk
                    lg_ps = plg.tile([P, 512], fp32, tag="lg_ps",
                                     name=f"lgps{ci}_{pi}")
                    for j in range(pi, pe):
                        # only the 64 expert columns are needed: slice the
                        # identity's moving operand (halves transpose cycles
                        # and folds in the col-group offset, exactly)
                        cb = 0 if j < nblk // 2 else 64
                        nc.tensor.transpose(
                            lg_ps[:, (j - pi) * 64:(j - pi + 1) * 64],
                            lt[:, j * P:(j + 1) * P], ident[:, cb:cb + 64],
                        )
                    for j in range(pi, pe):
                        s = s0 + j
                        seg = lg_ps[:, (j - pi) * 64:(j - pi) * 64 + 64]
                        segs[s] = seg
                        nc.vector.max(out=mx_acc[:, s * 8:(s + 1) * 8],
                                      in_=seg)
                        if not last:
                            nc.vector.max_index(
                                mi_acc[:, s * 8:(s + 1) * 8],
                                mx_acc[:, s * 8:(s + 1) * 8], seg,
                            )
                s1 = s0 + nblk
                nsg = nblk
                # weights path first: it only needs the max VALUES, so the
                # sigmoids + weight DMA (scalar ring) overlap the index
                # chain still running on the DVE
                delta = epool.tile([P, 16], fp32, tag="delta",
                                   name=f"delta{ci}")
                nc.vector.tensor_tensor(delta[:, :nsg], mx3[:, s0:s1, 1],
                                        mx3[:, s0:s1, 0],
                                        op=mybir.AluOpType.subtract)
                nc.scalar.activation(wv[:, s0:s1, 1], delta[:, :nsg],
                                     mybir.ActivationFunctionType.Sigmoid)
                nc.scalar.activation(wv[:, s0:s1, 0], delta[:, :nsg],
                                     mybir.ActivationFunctionType.Sigmoid,
                                     scale=-1.0)
                nc.scalar.dma_start(o_out.ap()[:, s0 * K:s1 * K],
                                    acc_all[:, s0 * K:s1 * K])
                # index chain for the final pass, then indices out (sync ring)
                lp = (nblk - 1) // 4 * 4
                for j in range(lp, nblk):
                    s = s0 + j
                    nc.vector.max_index(
                        mi_acc[:, s * 8:(s + 1) * 8],
                        mx_acc[:, s * 8:(s + 1) * 8], segs[s],
                    )
                nc.gpsimd.tensor_copy(
                    acc_all[:, NSEG * K + s0 * K: NSEG * K + s1 * K]
                    .rearrange("p (s k) -> p s k", k=K),
                    mi3[:, s0:s1, 0:K])
                nc.sync.dma_start(
                    o_out.ap()[:, NSEG * K + s0 * K:NSEG * K + s1 * K],
                    acc_all[:, NSEG * K + s0 * K:NSEG * K + s1 * K])

            # chunk 0: tokens 0:1024 from the A pieces
            pga0 = pacc.tile([64, 512], fp32, tag="gA", name="pga0")
            pgb0 = pacc.tile([P, 512], fp32, tag="gB", name="pgb0")
            emit_mm(0, 0, 1024, pga0, pgb0, range(NDT))
            emit_epilogue(0, 0, 1024, pga0, pgb0)

            # B region (tokens 1024:2048): the two 512-token halves pack
            # into the two PE column quadrants of ONE matmul pair per
            # d-tile (moving dim 512 keeps the PE at full efficiency), but
            # each half's accumulation stops at its own last piece so the
            # left half's epilogue overlaps the stream tail
            pga1 = pacc.tile([64, 512], fp32, tag="gA", name="pga1")
            pgb1 = pacc.tile([P, 512], fp32, tag="gB", name="pgb1")
            # A->B seam: fillers must depend on b0's DATA, otherwise the
            # Tile scheduler hoists them next to the warm-ups (v3 bug: the
            # whole B region then ran at the cold 1.2 GHz clock).  A chain
            # of b0-gated fillers re-warms the HAM right at the seam.
            # A->B seam: re-warm chain gated on b0 DATA (ident-moving fillers
            # get hoisted next to the warm-ups by the scheduler and the B
            # region then runs at the cold 1.2 GHz clock)
            b0t = pieces["b0"][4]
            for i in range(8):
                nc.tensor.matmul(warm[:], ident[:],
                                 b0t[:, i * P:(i + 1) * P],
                                 start=True, stop=True)
            # chunk1 serially (data lands 2 pieces before stream end, so
            # its epilogue overlaps the c pieces), then chunk2
            emit_mm_half(0, 1024, pga1[:, :512], range(0, 16), 15)
            emit_epilogue_half(1, 1024, pga1[:, :512], 0, 0)
            emit_mm_half(64, 1536, pgb1[64:128, :512], range(0, 16), 15)
            emit_epilogue_half(2, 1536, pgb1[64:128, :512], 64, 64)

    nc.compile()
    return nc


def _get_compiled():
    global _compiled
    if _compiled is None:
        _compiled = _build()
    return _compiled


def kernel(x, gate_w):
    from concourse.bass_utils import run_bass_kernel_spmd

    x = np.ascontiguousarray(np.asarray(x, dtype=np.float32))
    gate_w = np.ascontiguousarray(np.asarray(gate_w, dtype=np.float32))
    assert x.shape == (B, T, D) and gate_w.shape == (E, D)

    nc = _get_compiled()

    x_flat = x.reshape(B * T, D)
    # gate_w.T laid out [128, 16*64]: (p, dt*64+e) = gate_w[e, dt*128+p]
    gwl = np.ascontiguousarray(
        gate_w.T.reshape(NDT, P, E).transpose(1, 0, 2).reshape(P, NDT * E)
    )

    from concurrent.futures import ThreadPoolExecutor

    def shard(c):
        sl = x_flat[c * TOK_PER_CORE:(c + 1) * TOK_PER_CORE]  # [tok, D]
        out = np.empty((P, NDT * TOK_PER_CORE), dtype=np.float32)
        off = 0
        for (nm, d0, nd, t0, ntok) in PIECES:
            blk = sl[t0:t0 + ntok, d0 * P:(d0 + nd) * P]      # [ntok, nd*P]
            out[:, off:off + nd * ntok] = (
                blk.reshape(ntok, nd, P).transpose(2, 1, 0)
                .reshape(P, nd * ntok))
            off += nd * ntok
        return out

    with ThreadPoolExecutor(max_workers=N_CORES) as ex:
        shards = list(ex.map(shard, range(N_CORES)))

    in_maps = [{"xT": shards[c], "gwl": gwl} for c in range(N_CORES)]
    res = run_bass_kernel_spmd(nc, in_maps, list(range(N_CORES)))

    # device buffer is [P, 2*NSEG*K] u32: first half f32 weight bits,
    # second half indices; token = s*128 + p
    def unperm(buf):
        return buf.reshape(P, NSEG, K).transpose(1, 0, 2).reshape(
            TOK_PER_CORE, K)

    ws, idxs = [], []
    for c in range(N_CORES):
        o = res.results[c]["o"]
        ws.append(unperm(o[:, :NSEG * K].view(np.float32)))
        idxs.append(unperm(o[:, NSEG * K:]))
    weights = np.concatenate(ws, axis=0).reshape(B, T, K).astype(np.float32)
    indices = np.concatenate(idxs, axis=0).reshape(B, T, K).astype(np.int32)
    return weights, indices



# revision 14
# speedup vs baseline: 1.1576x; 1.1576x over previous
"""MoE router (top-2 gating) Trainium2 Bass kernel, SPMD over 8 NeuronCores.

Problem: x [4, 4096, 2048] f32, gate_w [64, 2048] f32.
  logits = x @ gate_w.T          -> [4, 4096, 64]
  scores, indices = top_k(logits, 2)
  weights = softmax(scores)      -> ([4, 4096, 2] f32, [4, 4096, 2] i32)

Sharding: data-parallel over tokens; each of the 8 cores gets 2048 tokens,
shipped transposed ([D, T]) so the contraction dim D lands on SBUF
partitions and the PE streams tokens as the moving operand of exact-fp32
matmuls (min top2/top3 logit gap on this input is 4e-7 — any reduced
precision flips indices, so the matmul must stay fp32).

Schedule (v3, from trace analysis):
  - ALL 8 input sub-DMAs issued first on the sync HWDGE ring; gw + output
    DMAs ride the scalar HWDGE ring (v1 stalled the input stream ~8us
    behind an output DMA waiting on softmax).
  - Every input piece keeps a >=256-token extent so DRAM rows stay >=1KB
    (fine token splits exploded the descriptor count in v2 and made the
    stream issue-bound: 2048-desc pieces took 8-12us to generate).
  - Pieces: 4x(4dt x tok[0:1024]), 2x(6dt x tok[1024:2048]),
    (4dt x tok[1024:1792]), (4dt x tok[1792:2048]).  The last piece is
    0.5 MiB so only 256 tokens' worth of top-k trails the stream.
  - Compute chunks [1024, 768, 256] slice into those pieces; chunk 1+2
    accumulate interleaved per d-tile as B pieces land.
  - PE warm-up matmuls lift the HAM clock gate before real work.
  - softmax(top2) via two ACT sigmoids (w1 = sig(d), w0 = sig(-d)).
  - index-half output DMAs go on the (idle) sync ring as soon as indices
    are compacted; weight halves follow sigmoids on the scalar ring.
"""
import sys

if "/opt/trn_rl_repo" not in sys.path:
    sys.path.insert(0, "/opt/trn_rl_repo")

import numpy as np

B, T, D, E, K = 4, 4096, 2048, 64, 2
N_CORES = 8
P = 128
NDT = D // P                      # 16 d-tiles
TOK_PER_CORE = B * T // N_CORES   # 2048
NSEG = TOK_PER_CORE // P          # 16 output segments of 128 tokens

# input pieces: (name, d0, ndt, t0, ntok), issued in this order
PIECES = [
    # geometric lead-in: the PE is rate-matched with the stream, so its
    # start lag rides through to the end of the kernel — tiny first
    # pieces let matmuls begin ~4.5us earlier
    ("a0", 0, 1, 0, 1024),
    ("a1", 1, 1, 0, 1024),
    ("a2", 2, 2, 0, 1024),
    ("a3", 4, 4, 0, 1024),
    ("a4", 8, 4, 0, 1024),
    ("a5", 12, 4, 0, 1024),
    ("b0", 0, 2, 1024, 1024),
    ("b1", 2, 2, 1024, 1024),
    ("b2", 4, 2, 1024, 1024),
    ("b3", 6, 2, 1024, 1024),
    ("b4", 8, 2, 1024, 1024),
    ("b5", 10, 2, 1024, 1024),
    ("b6", 12, 2, 1024, 512),
    ("b6b", 14, 2, 1024, 512),
    ("b7", 12, 2, 1536, 512),
    ("b7b", 14, 2, 1536, 512),
]
# compute chunks: (t0, ntok); chunk k completes when its last piece lands,
# staggered so each epilogue overlaps the remaining stream
CHUNKS = [(0, 1024), (1024, 512), (1536, 512)]
N_WARMUP = 10          # PE warm-up matmuls before the first real one

_compiled = None


def _build():
    import concourse.bacc as bacc
    import concourse.tile as tile
    from concourse import mybir
    from concourse.masks import make_identity

    nc = bacc.Bacc("TRN2", target_bir_lowering=False, debug=False,
                   num_devices=N_CORES)

    xT_in = nc.dram_tensor("xT", [D, TOK_PER_CORE], mybir.dt.float32,
                           kind="ExternalInput")
    gw_in = nc.dram_tensor("gwl", [P, NDT * E], mybir.dt.float32,
                           kind="ExternalInput")
    # single merged output: [:, 0:NSEG*K] = weight bits (f32), rest = indices
    o_out = nc.dram_tensor("o", [P, NSEG * K * 2], mybir.dt.uint32,
                           kind="ExternalOutput")

    fp32 = mybir.dt.float32

    with tile.TileContext(nc) as tc:
        with (
            tc.tile_pool(name="xpool", bufs=1) as xpool,
            tc.tile_pool(name="cpool", bufs=1) as cpool,
            tc.tile_pool(name="epool", bufs=2) as epool,
            tc.tile_pool(name="opool", bufs=1) as opool,
            tc.tile_pool(name="pacc", bufs=2, space="PSUM") as pacc,
            tc.tile_pool(name="plg", bufs=2, space="PSUM") as plg,
            tc.tile_pool(name="pwarm", bufs=1, space="PSUM") as pwarm,
        ):
            # ---- input stream: every sub-DMA issued first, sync ring ----
            xT_v = xT_in.ap().rearrange("(dt p) t -> p dt t", p=P)
            pieces = {}   # name -> (d0, ndt, t0, ntok, tile)
            for (nm, d0, nd, t0, ntok) in PIECES:
                xt = xpool.tile([P, nd * ntok], fp32,
                                tag=f"x_{nm}", name=f"xt_{nm}")
                nc.sync.dma_start(
                    xt[:].rearrange("p (dt t) -> p dt t", dt=nd),
                    xT_v[:, d0:d0 + nd, t0:t0 + ntok],
                )
                pieces[nm] = (d0, nd, t0, ntok, xt)

            def src_ap(dt, ct0, cn):
                """moving operand slice for d-tile dt, tokens [ct0, ct0+cn)"""
                for (d0, nd, t0, ntok, xt) in pieces.values():
                    if d0 <= dt < d0 + nd and t0 <= ct0 and ct0 + cn <= t0 + ntok:
                        base = (dt - d0) * ntok + (ct0 - t0)
                        return xt[:, base:base + cn]
                raise AssertionError((dt, ct0, cn))

            # ---- constants / one-time loads (scalar ring for gw) ----
            # split so the dt0-3 slice lands before the first matmul needs
            # it (scalar-ring packets interleave 1:1 with the input flood,
            # so the full 512KB otherwise gates the PE until ~13.3us)
            gw_sb = cpool.tile([P, NDT * E], fp32)
            nc.scalar.dma_start(gw_sb[:, 0:4 * E], gw_in.ap()[:, 0:4 * E])
            nc.scalar.dma_start(gw_sb[:, 4 * E:], gw_in.ap()[:, 4 * E:])
            ident = cpool.tile([P, P], fp32)
            make_identity(nc, ident[:])
            # warm the ACT sigmoid table early (overlaps input stream)
            scratch = cpool.tile([P, 1], fp32)
            nc.gpsimd.memset(scratch[:], 0.0)
            nc.scalar.activation(scratch[:], scratch[:],
                                 mybir.ActivationFunctionType.Sigmoid)

            # ---- PE warm-up: lift the HAM clock gate before real work ----
            warm = pwarm.tile([P, P], fp32, tag="warm", name="warm")
            for wi in range(N_WARMUP):
                nc.tensor.matmul(warm[:], ident[:], ident[:],
                                 start=True, stop=True)

            # ---- per-core accumulators ----
            mx_acc = opool.tile([P, NSEG * 8], fp32)
            mi_acc = opool.tile([P, NSEG * 8], mybir.dt.uint32)
            acc_all = opool.tile([P, NSEG * K * 2], mybir.dt.uint32)
            mx3 = mx_acc[:].rearrange("p (s k) -> p s k", k=8)
            wv = acc_all[:, 0:NSEG * K].bitcast(fp32).rearrange(
                "p (s k) -> p s k", k=K)
            mi3 = mi_acc[:].rearrange("p (s k) -> p s k", k=8)

            def emit_mm(ci, ct0, cn, pga, pgb, dts):
                half = cn // 2
                for dt in dts:
                    gsl = gw_sb[:, dt * E:(dt + 1) * E]
                    mmargs = dict(start=(dt == 0), stop=(dt == NDT - 1))
                    nc.tensor.matmul(pga[:, :half], gsl,
                                     src_ap(dt, ct0, half),
                                     tile_position=(0, 0), **mmargs)
                    nc.tensor.matmul(pgb[64:128, :half], gsl,
                                     src_ap(dt, ct0 + half, half),
                                     tile_position=(0, 64), **mmargs)

            def emit_mm_half(quad, ct0, pq, dts, stop_dt):
                # one 512-token half packed into PE column quadrant `quad`;
                # its accumulation group stops at its own last d-tile
                for dt in dts:
                    gsl = gw_sb[:, dt * E:(dt + 1) * E]
                    nc.tensor.matmul(
                        pq, gsl, src_ap(dt, ct0, 512),
                        tile_position=(0, quad),
                        start=(dt == 0), stop=(dt == stop_dt))

            def emit_epilogue_half(ci, ct0, pq, rowlo, cb):
                # epilogue for a 512-token half living in PSUM rows
                # [rowlo, rowlo+64); experts land at cols [cb, cb+64)
                s0 = ct0 // P
                lt = epool.tile([P, 1024], fp32, tag="lt", name=f"lt{ci}")
                cp = nc.vector.tensor_copy if rowlo == 0 else nc.scalar.copy
                cp(lt[rowlo:rowlo + 64, 0:512], pq)
                lg_ps = plg.tile([P, 512], fp32, tag="lg_ps",
                                 name=f"lgps{ci}")
                for j in range(4):
                    nc.tensor.transpose(
                        lg_ps[:, j * P:(j + 1) * P],
                        lt[:, j * P:(j + 1) * P], ident[:],
                    )
                for j in range(4):
                    s = s0 + j
                    nc.vector.max(
                        out=mx_acc[:, s * 8:(s + 1) * 8],
                        in_=lg_ps[:, j * P + cb: j * P + cb + 64])
                s1, nsg = s0 + 4, 4
                delta = epool.tile([P, 16], fp32, tag="delta",
                                   name=f"delta{ci}")
                nc.vector.tensor_tensor(delta[:, :nsg], mx3[:, s0:s1, 1],
                                        mx3[:, s0:s1, 0],
                                        op=mybir.AluOpType.subtract)
                nc.scalar.activation(wv[:, s0:s1, 1], delta[:, :nsg],
                                     mybir.ActivationFunctionType.Sigmoid)
                nc.scalar.activation(wv[:, s0:s1, 0], delta[:, :nsg],
                                     mybir.ActivationFunctionType.Sigmoid,
                                     scale=-1.0)
                nc.scalar.dma_start(o_out.ap()[:, s0 * K:s1 * K],
                                    acc_all[:, s0 * K:s1 * K])
                for j in range(4):
                    s = s0 + j
                    nc.vector.max_index(
                        mi_acc[:, s * 8:(s + 1) * 8],
                        mx_acc[:, s * 8:(s + 1) * 8],
                        lg_ps[:, j * P + cb: j * P + cb + 64])
                nc.gpsimd.tensor_copy(
                    acc_all[:, NSEG * K + s0 * K: NSEG * K + s1 * K]
                    .rearrange("p (s k) -> p s k", k=K),
                    mi3[:, s0:s1, 0:K])
                nc.sync.dma_start(
                    o_out.ap()[:, NSEG * K + s0 * K:NSEG * K + s1 * K],
                    acc_all[:, NSEG * K + s0 * K:NSEG * K + s1 * K])

            def emit_epilogue(ci, ct0, cn, pga, pgb):
                half = cn // 2
                nblk = cn // P
                s0 = ct0 // P
                # copy the two logits.T halves into token-aligned quadrants
                # (gpsimd can't read PSUM, so DVE + scalar)
                lt = epool.tile([P, 1024], fp32, tag="lt", name=f"lt{ci}")
                nc.vector.tensor_copy(lt[0:64, 0:half], pga[:, :half])
                nc.scalar.copy(lt[64:128, half:cn], pgb[64:128, :half])
                # back-transpose in passes of <=4 blocks (plg bank = 512 f32);
                # top-8 / top-8-index read the transposed PSUM directly
                segs = {}
                for pi in range(0, nblk, 4):
                    pe = min(pi + 4, nblk)
                    last = pe == nblk
                    lg_ps = plg.tile([P, 512], fp32, tag="lg_ps",
                                     name=f"lgps{ci}_{pi}")
                    for j in range(pi, pe):
                        # only the 64 expert columns are needed: slice the
                        # identity's moving operand (halves transpose cycles
                        # and folds in the col-group offset, exactly)
                        cb = 0 if j < nblk // 2 else 64
                        nc.tensor.transpose(
                            lg_ps[:, (j - pi) * 64:(j - pi + 1) * 64],
                            lt[:, j * P:(j + 1) * P], ident[:, cb:cb + 64],
                        )
                    for j in range(pi, pe):
                        s = s0 + j
                        seg = lg_ps[:, (j - pi) * 64:(j - pi) * 64 + 64]
                        segs[s] = seg
                        nc.vector.max(out=mx_acc[:, s * 8:(s + 1) * 8],
                                      in_=seg)
                        if not last:
                            nc.vector.max_index(
                                mi_acc[:, s * 8:(s + 1) * 8],
                                mx_acc[:, s * 8:(s + 1) * 8], seg,
                            )
                s1 = s0 + nblk
                nsg = nblk
                # weights path first: it only needs the max VALUES, so the
                # sigmoids + weight DMA (scalar ring) overlap the index
                # chain still running on the DVE
                delta = epool.tile([P, 16], fp32, tag="delta",
                                   name=f"delta{ci}")
                nc.vector.tensor_tensor(delta[:, :nsg], mx3[:, s0:s1, 1],
                                        mx3[:, s0:s1, 0],
                                        op=mybir.AluOpType.subtract)
                nc.scalar.activation(wv[:, s0:s1, 1], delta[:, :nsg],
                                     mybir.ActivationFunctionType.Sigmoid)
                nc.scalar.activation(wv[:, s0:s1, 0], delta[:, :nsg],
                                     mybir.ActivationFunctionType.Sigmoid,
                                     scale=-1.0)
                nc.scalar.dma_start(o_out.ap()[:, s0 * K:s1 * K],
                                    acc_all[:, s0 * K:s1 * K])
                # index chain for the final pass, then indices out (sync ring)
                lp = (nblk - 1) // 4 * 4
                for j in range(lp, nblk):
                    s = s0 + j
                    nc.vector.max_index(
                        mi_acc[:, s * 8:(s + 1) * 8],
                        mx_acc[:, s * 8:(s + 1) * 8], segs[s],
                    )
                nc.gpsimd.tensor_copy(
                    acc_all[:, NSEG * K + s0 * K: NSEG * K + s1 * K]
                    .rearrange("p (s k) -> p s k", k=K),
                    mi3[:, s0:s1, 0:K])
                nc.sync.dma_start(
                    o_out.ap()[:, NSEG * K + s0 * K:NSEG * K + s1 * K],
                    acc_all[:, NSEG * K + s0 * K:NSEG * K + s1 * K])

            # chunk 0: tokens 0:1024 from the A pieces
            pga0 = pacc.tile([64, 512], fp32, tag="gA", name="pga0")
            pgb0 = pacc.tile([P, 512], fp32, tag="gB", name="pgb0")
            emit_mm(0, 0, 1024, pga0, pgb0, range(NDT))
            emit_epilogue(0, 0, 1024, pga0, pgb0)

            # B region (tokens 1024:2048): the two 512-token halves pack
            # into the two PE column quadrants of ONE matmul pair per
            # d-tile (moving dim 512 keeps the PE at full efficiency), but
            # each half's accumulation stops at its own last piece so the
            # left half's epilogue overlaps the stream tail
            pga1 = pacc.tile([64, 512], fp32, tag="gA", name="pga1")
            pgb1 = pacc.tile([P, 512], fp32, tag="gB", name="pgb1")
            # A->B seam: the PE idles here waiting for b0 and the clock
            # gate cools; a few fillers (own accumulation group, emitted
            # OUTSIDE any open group) keep it hot
            for _ in range(4):
                nc.tensor.matmul(warm[:], ident[:], ident[:],
                                 start=True, stop=True)
            for dt in range(12):
                emit_mm_half(0, 1024, pga1[:, :512], [dt], 15)
                emit_mm_half(64, 1536, pgb1[64:128, :512], [dt], 15)
            emit_mm_half(0, 1024, pga1[:, :512], range(12, 16), 15)
            emit_epilogue_half(1, 1024, pga1[:, :512], 0, 0)
            emit_mm_half(64, 1536, pgb1[64:128, :512], range(12, 16), 15)
            emit_epilogue_half(2, 1536, pgb1[64:128, :512], 64, 64)

    nc.compile()
    return nc


def _get_compiled():
    global _compiled
    if _compiled is None:
        _compiled = _build()
    return _compiled


def kernel(x, gate_w):
    from concourse.bass_utils import run_bass_kernel_spmd

    x = np.ascontiguousarray(np.asarray(x, dtype=np.float32))
    gate_w = np.ascontiguousarray(np.asarray(gate_w, dtype=np.float32))
    assert x.shape == (B, T, D) and gate_w.shape == (E, D)

    nc = _get_compiled()

    x_flat = x.reshape(B * T, D)
    # gate_w.T laid out [128, 16*64]: (p, dt*64+e) = gate_w[e, dt*128+p]
    gwl = np.ascontiguousarray(
        gate_w.T.reshape(NDT, P, E).transpose(1, 0, 2).reshape(P, NDT * E)
    )

    from concurrent.futures import ThreadPoolExecutor

    def shard(c):
        sl = x_flat[c * TOK_PER_CORE:(c + 1) * TOK_PER_CORE]
        return np.ascontiguousarray(sl.T)  # [D, TOK_PER_CORE]

    with ThreadPoolExecutor(max_workers=N_CORES) as ex:
        shards = list(ex.map(shard, range(N_CORES)))

    in_maps = [{"xT": shards[c], "gwl": gwl} for c in range(N_CORES)]
    res = run_bass_kernel_spmd(nc, in_maps, list(range(N_CORES)))

    # device buffer is [P, 2*NSEG*K] u32: first half f32 weight bits,
    # second half indices; token = s*128 + p
    def unperm(buf):
        return buf.reshape(P, NSEG, K).transpose(1, 0, 2).reshape(
            TOK_PER_CORE, K)

    ws, idxs = [], []
    for c in range(N_CORES):
        o = res.results[c]["o"]
        ws.append(unperm(o[:, :NSEG * K].view(np.float32)))
        idxs.append(unperm(o[:, NSEG * K:]))
    weights = np.concatenate(ws, axis=0).reshape(B, T, K).astype(np.float32)
    indices = np.concatenate(idxs, axis=0).reshape(B, T, K).astype(np.int32)
    return weights, indices

